# revision 13
# baseline (speedup 1.0000x reference)
"""Trainium2 Bass kernel for the ExoplanetGNN heterograph message-passing net.

Self-contained: builds host-side edge grids, compiles one SPMD Bass program,
runs it on 8 NeuronCores via run_bass_kernel_spmd, reassembles the output.

Design:
 - dst-sharded edges: core c owns planet shard c and star shard c and all edges
   whose dst lands there; aggregation is complete per core (no reduce).
 - node feature tables (bf16, node-major [rows, 128]: 64 real feats + 64 pad
   so rows are 256B) are fully replicated per core; after each layer, shards
   are AllGathered into the next layer's table.
 - gathers use InstDMAGatherAnt (dma_gather): int16 indices, sign-extended
   against a sliced table base, one instruction per (4-chunk group x 64Ki src
   range) moving ~1-2K rows (994ns fixed SWDGE cost amortized).
 - per-128-edge tile: DVE one-hot (fp16 iota is_equal dst_rel) * (1/deg),
   TensorE segment matmul over the tile's dst window accumulating transposed
   aggregates [64 feat x 512 nodes] in PSUM.
 - per 512-node chunk: stacked rhs (two relations' aggregates for planets /
   aggregate+xT for stars), K=128/64 matmuls apply the SAGE linear layers,
   ScalarE fuses bias+ReLU, HWDGE dma-transpose produces node-major tiles for
   the table shard; a feat-major copy (xT) is kept for the self term.
 - layer 2 skips the star update and fuses the readout MLP on the planet path.
"""

import numpy as np
import ml_dtypes

import concourse.bass as bass
import concourse.bacc as bacc
import concourse.mybir as mybir
import concourse.tile as tile
from concourse.bass_utils import run_bass_kernel_spmd

BF16 = ml_dtypes.bfloat16
BF = mybir.dt.bfloat16
F16 = mybir.dt.float16
F32 = mybir.dt.float32
I16 = mybir.dt.int16
AF = mybir.ActivationFunctionType
ALU = mybir.AluOpType

C = 8            # cores
N_SWDGE_Q = 4
BLK = 128        # dst nodes per block
CHUNK_BLKS = 4   # node blocks per compute chunk (512 dst = one-hot window)
SEG = CHUNK_BLKS * BLK
GRP = 4          # chunks per gather group
RANGE = 65536    # int16 sign-extended index span
RBASE = 32768
EW = 128         # table row width in bf16 elems (256B rows)


class Cfg:
    def __init__(self, np_=500000, ns_=200000, fp=32, fs=16, h=64, l=3):
        self.NP, self.NS, self.FP, self.FS, self.H, self.L = np_, ns_, fp, fs, h, l
        assert np_ % C == 0 and ns_ % C == 0
        self.SP, self.SS = np_ // C, ns_ // C
        self.PB = -(-self.SP // BLK)
        self.SB = -(-self.SS // BLK)
        self.NPP, self.NSP = self.PB * BLK, self.SB * BLK
        self.NPT, self.NST = C * self.NPP, C * self.NSP


def _chunks(nblocks):
    out = []
    b = 0
    while b < nblocks:
        nb = min(CHUNK_BLKS, nblocks - b)
        out.append((b, nb))
        b += nb
    return out


def _prep_rel2(src, dst, src_shard, src_pad, dst_shard, nblocks, n_src_rows):
    """Per-core (chunk x src-range) cell grid for dma_gather aggregation.

    Returns (ix_wrap [C,128,8T] i16, dr [C,128,T] f32, w [C,128,T] f32, grid).
    Tile t slot p holds one edge: dr = dst offset within its 512-chunk,
    w = 1/deg(dst), ix = s_pad - base(range). Pad slots: dr=-1, w=0, ix=0.
    grid: T, ngrp, groups [(t_lo,t_hi)], instrs [[(base,t0,nt)]], chunk_tiles
    [[(t,lo,hi)]].
    """
    src = np.asarray(src, np.int64)
    dst = np.asarray(dst, np.int64)
    core = dst // dst_shard
    loc = dst - core * dst_shard
    nch = -(-(nblocks * BLK) // SEG)
    ch = loc // SEG
    drel = loc - ch * SEG
    NR = -(-n_src_rows // RANGE)
    bases = np.minimum(RBASE + RANGE * np.arange(NR), n_src_rows - RBASE)
    s_core = src // src_shard
    s_pad = s_core * src_pad + (src - s_core * src_shard)
    k = s_pad >> 16
    ngrp = -(-nch // GRP)
    g = ch // GRP
    cc = ch - g * GRP

    cellid = ((core * ngrp + g) * NR + k) * GRP + cc
    ncell = ngrp * NR * GRP
    cnt = np.bincount(cellid, minlength=C * ncell).reshape(C, ngrp, NR, GRP)
    nt_cell = -(-cnt.max(axis=0) // BLK)  # [ngrp, NR, GRP]
    for gg in range(ngrp):
        for c2 in range(min(GRP, nch - gg * GRP)):
            if nt_cell[gg, :, c2].sum() == 0:
                nt_cell[gg, 0, c2] = 1
    flat = nt_cell.reshape(-1)
    tb_flat = np.concatenate([[0], np.cumsum(flat)]).astype(np.int64)
    T = int(tb_flat[-1])
    tile_of_cell = tb_flat[:-1].reshape(ngrp, NR, GRP)

    order = np.lexsort((drel, cellid))
    cid_s = cellid[order]
    first = np.searchsorted(cid_s, cid_s, side="left")
    pos = np.arange(len(order)) - first
    tb_e = tile_of_cell[g[order], k[order], cc[order]]
    t_idx = tb_e + pos // BLK
    p_idx = pos % BLK
    c_idx = core[order]

    deg = np.bincount(core * dst_shard + loc, minlength=C * dst_shard)
    w_e = (1.0 / np.maximum(deg, 1))[core * dst_shard + loc]
    idxv = s_pad - bases[k]
    assert idxv.min() >= -RBASE and idxv.max() < RBASE

    dr_a = np.full((C, BLK, T), -1.0, np.float32)
    w_a = np.zeros((C, BLK, T), np.float32)
    ix_a = np.zeros((C, BLK, T), np.int16)
    dr_a[c_idx, p_idx, t_idx] = drel[order]
    w_a[c_idx, p_idx, t_idx] = w_e[order]
    ix_a[c_idx, p_idx, t_idx] = idxv[order]

    # tile dst windows (union over cores); empty tiles -> [0,1)
    real = w_a > 0
    drm = np.where(real, dr_a, 1e9).min(axis=(0, 1))
    drx = np.where(real, dr_a, -1e9).max(axis=(0, 1))
    lo_t = np.where(drm > drx, 0, drm).astype(np.int64)
    hi_t = np.where(drm > drx, 1, drx + 1).astype(np.int64)

    groups, instrs = [], []
    for gg in range(ngrp):
        t_lo = int(tile_of_cell[gg, 0, 0])
        ins_g = []
        for kk in range(NR):
            nt_gk = int(nt_cell[gg, kk].sum())
            if nt_gk:
                ins_g.append((int(bases[kk]), int(tile_of_cell[gg, kk, 0]), nt_gk))
                # the HW skips a trailing run of negative idxs: make sure the
                # final slot of each instruction stream is non-negative
                tf = int(tile_of_cell[gg, kk, 0]) + nt_gk - 1
                for c in range(C):
                    if ix_a[c, BLK - 1, tf] < 0:
                        p = int(np.argmax(ix_a[c, :, tf] >= 0))
                        assert ix_a[c, p, tf] >= 0, "all-negative final tile"
                        for arr in (ix_a, dr_a, w_a):
                            tmp = arr[c, BLK - 1, tf].copy()
                            arr[c, BLK - 1, tf] = arr[c, p, tf]
                            arr[c, p, tf] = tmp
        t_hi = t_lo + sum(n for _, _, n in ins_g)
        groups.append((t_lo, t_hi))
        instrs.append(ins_g)

    chunk_tiles = [[] for _ in range(nch)]
    for gg in range(ngrp):
        for kk in range(NR):
            for c2 in range(GRP):
                ci = gg * GRP + c2
                if ci >= nch:
                    continue
                t0 = int(tile_of_cell[gg, kk, c2])
                for t in range(t0, t0 + int(nt_cell[gg, kk, c2])):
                    chunk_tiles[ci].append((t, int(lo_t[t]), int(hi_t[t])))

    # wrap-16 index layout: tile t slot p -> [16a + p%16, 8t + p//16]
    wr = ix_a.transpose(0, 2, 1).reshape(C, T, 8, 16)
    wr = wr.transpose(0, 3, 1, 2).reshape(C, 16, T * 8)
    ix_wrap = np.tile(wr, (1, 8, 1))

    grid = dict(T=T, ngrp=ngrp, groups=groups, instrs=instrs,
                chunk_tiles=chunk_tiles)
    return ix_wrap, dr_a, w_a, grid


def build(cfg, grids, b2val):
    H, FP, FS, L = cfg.H, cfg.FP, cfg.FS, cfg.L
    nc = bacc.Bacc(None, target_bir_lowering=False, num_devices=C,
                   num_swdge_queues=N_SWDGE_Q)

    def param(name, shape, dt):
        return nc.declare_dram_parameter(name, shape, dt, isOutput=False)

    xpt = param("xpt", [FP, cfg.NPP], BF)
    xst = param("xst", [FS, cfg.NSP], BF)
    eparams = {}
    for r in ("orb", "hst", "sib"):
        T = grids[r]["T"]
        eparams[r] = (
            param(f"{r}_ix", [BLK, 8 * T], I16),
            param(f"{r}_dr", [BLK, T], F32),
            param(f"{r}_w", [BLK, T], F32),
        )
    iota_p = param("iota", [128, SEG], F16)
    wp_p = param("wp", [FP, H], BF)
    bp_p = param("bp", [H, 1], F32)
    ws_p = param("ws", [FS, H], BF)
    bs_p = param("bs", [H, 1], F32)
    wstack_s_p = param("wstack_s", [L, 2 * H, H], BF)
    wstack_p_p = param("wstack_p", [L, 2 * H, H], BF)
    wr_p_p = param("wr_p", [L, H, H], BF)
    bias_s_p = param("bias_s", [L, H, 1], F32)
    bias_p_p = param("bias_p", [L, H, 1], F32)
    w1_p = param("w1", [H, H // 2], BF)
    b1_p = param("b1", [H // 2, 1], F32)
    w2_p = param("w2", [H // 2, 1], BF)
    out_p = nc.declare_dram_parameter("out", [1, cfg.NPP], F32, isOutput=True)

    pchunks = _chunks(cfg.PB)
    schunks = _chunks(cfg.SB)
    qctr = [0]

    def next_q():
        qctr[0] += 1
        return qctr[0] % N_SWDGE_Q

    with tile.TileContext(nc) as tc:
        with (
            tc.tile_pool(name="const", bufs=1) as cp,
            tc.tile_pool(name="dram", bufs=1, space="DRAM") as dp,
            tc.tile_pool(name="idx", bufs=2) as ip,
            tc.tile_pool(name="msg", bufs=2) as mp,
            tc.tile_pool(name="sel", bufs=16) as selp,
            tc.tile_pool(name="work", bufs=4) as wkp,
            tc.tile_pool(name="psum", bufs=1, space="PSUM") as pp,
        ):
            # ---- persistent DRAM state ----
            hp_tab = [
                dp.tile([cfg.NPT, EW], BF, addr_space="Shared",
                        tag=f"hp_tab{i}", name=f"hp_tab{i}")
                for i in range(L)
            ]
            hs_tab = [
                dp.tile([cfg.NST, EW], BF, addr_space="Shared",
                        tag=f"hs_tab{i}", name=f"hs_tab{i}")
                for i in range(L)
            ]
            xpT = [dp.tile([H, cfg.NPP], BF, tag=f"xpT{i}", name=f"xpT{i}")
                   for i in range(2)]
            xsT = [dp.tile([H, cfg.NSP], BF, tag=f"xsT{i}", name=f"xsT{i}")
                   for i in range(2)]
            hp_shard = dp.tile([cfg.NPP, EW], BF, tag="hp_shard")
            hs_shard = dp.tile([cfg.NSP, EW], BF, tag="hs_shard")

            # ---- consts ----
            iota_t = cp.tile([128, SEG], F16, tag="iota")
            nc.sync.dma_start(out=iota_t[:], in_=iota_p[:, :])
            wp_t = cp.tile([FP, H], BF, tag="wp")
            nc.sync.dma_start(out=wp_t[:], in_=wp_p[:, :])
            ws_t = cp.tile([FS, H], BF, tag="ws")
            nc.sync.dma_start(out=ws_t[:], in_=ws_p[:, :])
            bp_t = cp.tile([H, 1], F32, tag="bp")
            nc.sync.dma_start(out=bp_t[:], in_=bp_p[:, :])
            bs_t = cp.tile([H, 1], F32, tag="bs")
            nc.sync.dma_start(out=bs_t[:], in_=bs_p[:, :])
            w1_t = cp.tile([H, H // 2], BF, tag="w1")
            nc.sync.dma_start(out=w1_t[:], in_=w1_p[:, :])
            b1_t = cp.tile([H // 2, 1], F32, tag="b1")
            nc.sync.dma_start(out=b1_t[:], in_=b1_p[:, :])
            w2_t = cp.tile([H // 2, 1], BF, tag="w2")
            nc.sync.dma_start(out=w2_t[:], in_=w2_p[:, :])
            wstack_s_t, wstack_p_t, wr_p_t, bias_s_t, bias_p_t = [], [], [], [], []
            for l in range(L):
                t = cp.tile([2 * H, H], BF, tag=f"wss{l}")
                nc.sync.dma_start(out=t[:], in_=wstack_s_p[l, :, :])
                wstack_s_t.append(t)
                t = cp.tile([2 * H, H], BF, tag=f"wsp{l}")
                nc.sync.dma_start(out=t[:], in_=wstack_p_p[l, :, :])
                wstack_p_t.append(t)
                t = cp.tile([H, H], BF, tag=f"wrp{l}")
                nc.sync.dma_start(out=t[:], in_=wr_p_p[l, :, :])
                wr_p_t.append(t)
                t = cp.tile([H, 1], F32, tag=f"bss{l}")
                nc.sync.dma_start(out=t[:], in_=bias_s_p[l, :, :])
                bias_s_t.append(t)
                t = cp.tile([H, 1], F32, tag=f"bsp{l}")
                nc.sync.dma_start(out=t[:], in_=bias_p_p[l, :, :])
                bias_p_t.append(t)

            def allgather(shard, tab):
                nc.gpsimd.collective_compute(
                    "AllGather",
                    ALU.bypass,
                    replica_groups=[list(range(C))],
                    ins=[shard[:, :]],
                    outs=[tab[:, :]],
                )

            def write_out_chunk(ob, c0, cw, nb, shard, xT_next):
                nc.sync.dma_start(out=xT_next[:, c0 : c0 + cw], in_=ob[:, :cw])
                for bi in range(nb):
                    eng = nc.sync if bi % 2 == 0 else nc.scalar
                    nm = wkp.tile([128, H], BF, tag="nm", bufs=6)
                    eng.dma_start_transpose(
                        out=nm[:], in_=ob[:, bi * 128 : (bi + 1) * 128]
                    )
                    r0 = c0 + bi * 128
                    eng.dma_start(out=shard[r0 : r0 + 128, 0:H], in_=nm[:])

            def load_group(rel, gg, table):
                """Load dr/w/ix spans and gather the group's messages."""
                gr = grids[rel]
                t_lo, t_hi = gr["groups"][gg]
                n = t_hi - t_lo
                ixp, drp, wp_ = eparams[rel]
                dr_t = ip.tile([BLK, n], F32, tag=f"{rel}_dr")
                nc.sync.dma_start(out=dr_t[:], in_=drp[:, t_lo:t_hi])
                w_t = ip.tile([BLK, n], F32, tag=f"{rel}_w")
                nc.sync.dma_start(out=w_t[:], in_=wp_[:, t_lo:t_hi])
                ix_t = ip.tile([BLK, 8 * n], I16, tag=f"{rel}_ix")
                nc.sync.dma_start(out=ix_t[:], in_=ixp[:, 8 * t_lo : 8 * t_hi])
                msg = mp.tile([128, n, EW], BF, tag=f"{rel}_msg", bufs=2)
                for (base, t0, nt) in gr["instrs"][gg]:
                    o = t0 - t_lo
                    nc.gpsimd.dma_gather(
                        out_ap=msg[:, o : o + nt, :],
                        in_ap=table[base:, :],
                        idxs_ap=ix_t[:, 8 * o : 8 * (o + nt)],
                        num_idxs=nt * BLK,
                        num_idxs_reg=nt * BLK,
                        elem_size=EW,
                        single_packet=(nt * BLK <= 1024),
                        queue_num=next_q(),
                    )
                return (dr_t, w_t, msg, t_lo, gr)

            def emit_agg(buf, ci, cw, agg):
                dr_t, w_t, msg, t_lo, gr = buf
                tiles = gr["chunk_tiles"][ci]
                ntl = len(tiles)
                for j, (gt, lo, hi) in enumerate(tiles):
                    if j == 0:
                        lo, hi = 0, cw
                    wdt = hi - lo
                    o = gt - t_lo
                    sel = selp.tile([128, SEG], BF, tag="sel", name="sel")
                    nc.vector.tensor_scalar(
                        out=sel[:, :wdt],
                        in0=iota_t[:, lo:hi],
                        scalar1=dr_t[:, o : o + 1],
                        scalar2=w_t[:, o : o + 1],
                        op0=ALU.is_equal,
                        op1=ALU.mult,
                    )
                    nc.tensor.matmul(
                        out=agg[:, lo:hi],
                        lhsT=msg[:, o : o + 1, 0:H].opt(),
                        rhs=sel[:, :wdt],
                        start=(j == 0),
                        stop=(j == ntl - 1),
                    )

            # =================== input projection ===================
            for (b0, nb) in pchunks:
                cw = nb * BLK
                c0 = b0 * BLK
                xp = wkp.tile([FP, SEG], BF, tag="xp")
                nc.sync.dma_start(out=xp[:, :cw], in_=xpt[:, c0 : c0 + cw])
                po = pp.tile([H, SEG], F32, tag="out", bufs=2)
                nc.tensor.matmul(
                    out=po[:, :cw], lhsT=wp_t[:], rhs=xp[:, :cw],
                    start=True, stop=True
                )
                ob = wkp.tile([H, SEG], BF, tag="ob")
                nc.scalar.activation(
                    out=ob[:, :cw], in_=po[:, :cw], func=AF.Relu,
                    bias=bp_t[:], scale=1.0
                )
                write_out_chunk(ob, c0, cw, nb, hp_shard, xpT[0])
            for (b0, nb) in schunks:
                cw = nb * BLK
                c0 = b0 * BLK
                xs = wkp.tile([FS, SEG], BF, tag="xs")
                nc.sync.dma_start(out=xs[:, :cw], in_=xst[:, c0 : c0 + cw])
                po = pp.tile([H, SEG], F32, tag="out", bufs=2)
                nc.tensor.matmul(
                    out=po[:, :cw], lhsT=ws_t[:], rhs=xs[:, :cw],
                    start=True, stop=True
                )
                ob = wkp.tile([H, SEG], BF, tag="ob")
                nc.scalar.activation(
                    out=ob[:, :cw], in_=po[:, :cw], func=AF.Relu,
                    bias=bs_t[:], scale=1.0
                )
                write_out_chunk(ob, c0, cw, nb, hs_shard, xsT[0])
            allgather(hp_shard, hp_tab[0])
            allgather(hs_shard, hs_tab[0])

            # =================== SAGE layers ===================
            for l in range(L):
                rp, wpar = l % 2, (l + 1) % 2
                rv, wv = l, l + 1
                # ---- stars (skip at last layer: no consumer) ----
                if l < L - 1:
                    for gg in range(grids["orb"]["ngrp"]):
                        buf_o = load_group("orb", gg, hp_tab[rv])
                        for ci in range(gg * GRP,
                                        min((gg + 1) * GRP, len(schunks))):
                            b0, nb = schunks[ci]
                            cw = nb * BLK
                            c0 = b0 * BLK
                            agg = pp.tile([H, SEG], F32, tag="agg_a", bufs=2)
                            emit_agg(buf_o, ci, cw, agg)
                            stacked = wkp.tile([2 * H, SEG], BF, tag="stacked")
                            nc.scalar.activation(
                                out=stacked[0:H, :cw], in_=agg[:, :cw],
                                func=AF.Copy
                            )
                            nc.sync.dma_start(
                                out=stacked[H : 2 * H, :cw],
                                in_=xsT[rp][:, c0 : c0 + cw]
                            )
                            po = pp.tile([H, SEG], F32, tag="out", bufs=2)
                            nc.tensor.matmul(
                                out=po[:, :cw],
                                lhsT=wstack_s_t[l][:],
                                rhs=stacked[:, :cw],
                                start=True,
                                stop=True,
                            )
                            ob = wkp.tile([H, SEG], BF, tag="ob")
                            nc.scalar.activation(
                                out=ob[:, :cw], in_=po[:, :cw], func=AF.Relu,
                                bias=bias_s_t[l][:], scale=1.0,
                            )
                            write_out_chunk(ob, c0, cw, nb, hs_shard, xsT[wpar])
                # ---- planets ----
                for gg in range(grids["sib"]["ngrp"]):
                    buf_h = load_group("hst", gg, hs_tab[rv])
                    buf_s = load_group("sib", gg, hp_tab[rv])
                    for ci in range(gg * GRP, min((gg + 1) * GRP, len(pchunks))):
                        b0, nb = pchunks[ci]
                        cw = nb * BLK
                        c0 = b0 * BLK
                        agg_h = pp.tile([H, SEG], F32, tag="agg_a", bufs=2)
                        emit_agg(buf_h, ci, cw, agg_h)
                        agg_s = pp.tile([H, SEG], F32, tag="agg_b", bufs=2)
                        emit_agg(buf_s, ci, cw, agg_s)
                        stacked = wkp.tile([2 * H, SEG], BF, tag="stacked")
                        nc.scalar.activation(
                            out=stacked[0:H, :cw], in_=agg_h[:, :cw], func=AF.Copy
                        )
                        nc.scalar.activation(
                            out=stacked[H : 2 * H, :cw], in_=agg_s[:, :cw],
                            func=AF.Copy
                        )
                        xt = wkp.tile([H, SEG], BF, tag="xt")
                        nc.sync.dma_start(
                            out=xt[:, :cw], in_=xpT[rp][:, c0 : c0 + cw]
                        )
                        po = pp.tile([H, SEG], F32, tag="out", bufs=2)
                        nc.tensor.matmul(
                            out=po[:, :cw],
                            lhsT=wstack_p_t[l][:],
                            rhs=stacked[:, :cw],
                            start=True,
                            stop=False,
                        )
                        nc.tensor.matmul(
                            out=po[:, :cw], lhsT=wr_p_t[l][:], rhs=xt[:, :cw],
                            start=False, stop=True,
                        )
                        ob = wkp.tile([H, SEG], BF, tag="ob")
                        nc.scalar.activation(
                            out=ob[:, :cw], in_=po[:, :cw], func=AF.Relu,
                            bias=bias_p_t[l][:], scale=1.0,
                        )
                        if l < L - 1:
                            write_out_chunk(ob, c0, cw, nb, hp_shard, xpT[wpar])
                        else:
                            pr = pp.tile([H // 2, SEG], F32, tag="r1", bufs=1)
                            nc.tensor.matmul(
                                out=pr[:, :cw], lhsT=w1_t[:], rhs=ob[:, :cw],
                                start=True, stop=True
                            )
                            r1 = wkp.tile([H // 2, SEG], BF, tag="r1sb")
                            nc.scalar.activation(
                                out=r1[:, :cw], in_=pr[:, :cw], func=AF.Relu,
                                bias=b1_t[:], scale=1.0,
                            )
                            py = pp.tile([1, SEG], F32, tag="y", bufs=1)
                            nc.tensor.matmul(
                                out=py[:, :cw], lhsT=w2_t[:], rhs=r1[:, :cw],
                                start=True, stop=True
                            )
                            ysb = wkp.tile([1, SEG], F32, tag="ysb")
                            nc.vector.tensor_scalar_add(
                                out=ysb[:, :cw], in0=py[:, :cw],
                                scalar1=float(b2val)
                            )
                            nc.sync.dma_start(
                                out=out_p[0:1, c0 : c0 + cw], in_=ysb[:, :cw]
                            )
                if l < L - 1:
                    allgather(hp_shard, hp_tab[wv])
                    allgather(hs_shard, hs_tab[wv])

    nc.finalize()
    return nc


def _prep_all(inputs, cfg):
    f32 = np.float32
    xp = np.asarray(inputs["x_planet"], f32)
    xs = np.asarray(inputs["x_star"], f32)
    Wp = np.asarray(inputs["Wp"], f32)
    bp = np.asarray(inputs["bp"], f32)
    Ws = np.asarray(inputs["Ws"], f32)
    bs = np.asarray(inputs["bs"], f32)
    Wl = np.asarray(inputs["Wl"], f32)
    bl = np.asarray(inputs["bl"], f32)
    Wr = np.asarray(inputs["Wr"], f32)
    W1 = np.asarray(inputs["W1"], f32)
    b1 = np.asarray(inputs["b1"], f32)
    W2 = np.asarray(inputs["W2"], f32)
    b2 = np.asarray(inputs["b2"], f32)

    rels = {}
    grids = {}
    rels["orb"] = _prep_rel2(inputs["orbits_src"], inputs["orbits_dst"],
                             cfg.SP, cfg.NPP, cfg.SS, cfg.SB, cfg.NPT)
    rels["hst"] = _prep_rel2(inputs["hosts_src"], inputs["hosts_dst"],
                             cfg.SS, cfg.NSP, cfg.SP, cfg.PB, cfg.NST)
    rels["sib"] = _prep_rel2(inputs["sib_src"], inputs["sib_dst"],
                             cfg.SP, cfg.NPP, cfg.SP, cfg.PB, cfg.NPT)
    for name in ("orb", "hst", "sib"):
        grids[name] = rels[name][3]

    L, H = cfg.L, cfg.H
    wstack_s = np.stack([np.concatenate([Wl[l, 0], Wr[l, 0]], 0) for l in range(L)])
    wstack_p = np.stack(
        [np.concatenate([0.5 * Wl[l, 1], 0.5 * Wl[l, 2]], 0) for l in range(L)]
    )
    wr_p = np.stack([0.5 * (Wr[l, 1] + Wr[l, 2]) for l in range(L)])
    bias_s = np.stack([bl[l, 0][:, None] for l in range(L)])
    bias_p = np.stack([0.5 * (bl[l, 1] + bl[l, 2])[:, None] for l in range(L)])
    iota = np.tile(np.arange(SEG, dtype=np.float16), (128, 1))

    common = {
        "iota": iota,
        "wp": Wp.astype(BF16), "bp": bp[:, None],
        "ws": Ws.astype(BF16), "bs": bs[:, None],
        "wstack_s": wstack_s.astype(BF16), "wstack_p": wstack_p.astype(BF16),
        "wr_p": wr_p.astype(BF16),
        "bias_s": bias_s, "bias_p": bias_p,
        "w1": W1.astype(BF16), "b1": b1[:, None], "w2": W2.astype(BF16),
    }
    in_maps = []
    for c in range(C):
        xpt_c = np.zeros((cfg.FP, cfg.NPP), BF16)
        xpt_c[:, : cfg.SP] = xp[c * cfg.SP : (c + 1) * cfg.SP].T.astype(BF16)
        xst_c = np.zeros((cfg.FS, cfg.NSP), BF16)
        xst_c[:, : cfg.SS] = xs[c * cfg.SS : (c + 1) * cfg.SS].T.astype(BF16)
        m = dict(common)
        m["xpt"] = xpt_c
        m["xst"] = xst_c
        for name in ("orb", "hst", "sib"):
            ix, dr, w, _ = rels[name]
            m[f"{name}_ix"] = ix[c]
            m[f"{name}_dr"] = dr[c]
            m[f"{name}_w"] = w[c]
        in_maps.append(m)
    return in_maps, grids, float(b2[0])


LAST_RESULT = None


def kernel(_cfg=None, _trace=False, **inputs):
    global LAST_RESULT
    cfg = _cfg or Cfg()
    in_maps, grids, b2val = _prep_all(inputs, cfg)
    nc = build(cfg, grids, b2val)
    res = run_bass_kernel_spmd(nc, in_maps, list(range(C)), trace=_trace)
    LAST_RESULT = res
    out = np.concatenate(
        [res.results[c]["out"][0, : cfg.SP] for c in range(C)]
    ).astype(np.float32)
    return out


# revision 19
# speedup vs baseline: 1.0469x; 1.0469x over previous
"""Trainium2 Bass kernel for the ExoplanetGNN heterograph message-passing net.

Self-contained: builds host-side edge grids, compiles one SPMD Bass program,
runs it on 8 NeuronCores via run_bass_kernel_spmd, reassembles the output.

Design:
 - dst-sharded edges: core c owns planet shard c and star shard c and all edges
   whose dst lands there; aggregation is complete per core (no reduce).
 - node feature tables (bf16, node-major [rows, 128]: 64 real feats + 64 pad
   so rows are 256B) are fully replicated per core; after each layer, shards
   are AllGathered into the next layer's table.
 - gathers use InstDMAGatherAnt (dma_gather): int16 indices, sign-extended
   against a sliced table base, one instruction per (4-chunk group x 64Ki src
   range) moving ~1-2K rows (994ns fixed SWDGE cost amortized).
 - per-128-edge tile: DVE one-hot (fp16 iota is_equal dst_rel) * (1/deg),
   TensorE segment matmul over the tile's dst window accumulating transposed
   aggregates [64 feat x 512 nodes] in PSUM.
 - per 512-node chunk: stacked rhs (two relations' aggregates for planets /
   aggregate+xT for stars), K=128/64 matmuls apply the SAGE linear layers,
   ScalarE fuses bias+ReLU, HWDGE dma-transpose produces node-major tiles for
   the table shard; a feat-major copy (xT) is kept for the self term.
 - layer 2 skips the star update and fuses the readout MLP on the planet path.
"""

import numpy as np
import ml_dtypes

import concourse.bass as bass
import concourse.bacc as bacc
import concourse.mybir as mybir
import concourse.tile as tile
from concourse.bass_utils import run_bass_kernel_spmd

BF16 = ml_dtypes.bfloat16
BF = mybir.dt.bfloat16
F16 = mybir.dt.float16
F32 = mybir.dt.float32
I16 = mybir.dt.int16
AF = mybir.ActivationFunctionType
ALU = mybir.AluOpType

C = 8            # cores
N_SWDGE_Q = 4
BLK = 128        # dst nodes per block
CHUNK_BLKS = 4   # node blocks per compute chunk (512 dst = one-hot window)
SEG = CHUNK_BLKS * BLK
GRP = 4          # chunks per gather group
RANGE = 65536    # int16 sign-extended index span
RBASE = 32768
EW = 128         # table row width in bf16 elems (256B rows)


class Cfg:
    def __init__(self, np_=500000, ns_=200000, fp=32, fs=16, h=64, l=3):
        self.NP, self.NS, self.FP, self.FS, self.H, self.L = np_, ns_, fp, fs, h, l
        assert np_ % C == 0 and ns_ % C == 0
        self.SP, self.SS = np_ // C, ns_ // C
        self.PB = -(-self.SP // BLK)
        self.SB = -(-self.SS // BLK)
        self.NPP, self.NSP = self.PB * BLK, self.SB * BLK
        self.NPT, self.NST = C * self.NPP, C * self.NSP


def _chunks(nblocks):
    out = []
    b = 0
    while b < nblocks:
        nb = min(CHUNK_BLKS, nblocks - b)
        out.append((b, nb))
        b += nb
    return out


def _prep_rel2(src, dst, src_shard, src_pad, dst_shard, nblocks, n_src_rows):
    """Per-core (chunk x src-range) cell grid for dma_gather aggregation.

    Returns (ix_wrap [C,128,8T] i16, dr [C,128,T] f32, w [C,128,T] f32, grid).
    Tile t slot p holds one edge: dr = dst offset within its 512-chunk,
    w = 1/deg(dst), ix = s_pad - base(range). Pad slots: dr=-1, w=0, ix=0.
    grid: T, ngrp, groups [(t_lo,t_hi)], instrs [[(base,t0,nt)]], chunk_tiles
    [[(t,lo,hi)]].
    """
    src = np.asarray(src, np.int64)
    dst = np.asarray(dst, np.int64)
    core = dst // dst_shard
    loc = dst - core * dst_shard
    nch = -(-(nblocks * BLK) // SEG)
    ch = loc // SEG
    drel = loc - ch * SEG
    NR = -(-n_src_rows // RANGE)
    bases = np.minimum(RBASE + RANGE * np.arange(NR), n_src_rows - RBASE)
    s_core = src // src_shard
    s_pad = s_core * src_pad + (src - s_core * src_shard)
    k = s_pad >> 16
    ngrp = -(-nch // GRP)
    g = ch // GRP
    cc = ch - g * GRP

    cellid = ((core * ngrp + g) * NR + k) * GRP + cc
    ncell = ngrp * NR * GRP
    cnt = np.bincount(cellid, minlength=C * ncell).reshape(C, ngrp, NR, GRP)
    nt_cell = -(-cnt.max(axis=0) // BLK)  # [ngrp, NR, GRP]
    for gg in range(ngrp):
        for c2 in range(min(GRP, nch - gg * GRP)):
            if nt_cell[gg, :, c2].sum() == 0:
                nt_cell[gg, 0, c2] = 1
    flat = nt_cell.reshape(-1)
    tb_flat = np.concatenate([[0], np.cumsum(flat)]).astype(np.int64)
    T = int(tb_flat[-1])
    tile_of_cell = tb_flat[:-1].reshape(ngrp, NR, GRP)

    order = np.lexsort((drel, cellid))
    cid_s = cellid[order]
    first = np.searchsorted(cid_s, cid_s, side="left")
    pos = np.arange(len(order)) - first
    tb_e = tile_of_cell[g[order], k[order], cc[order]]
    t_idx = tb_e + pos // BLK
    p_idx = pos % BLK
    c_idx = core[order]

    deg = np.bincount(core * dst_shard + loc, minlength=C * dst_shard)
    w_e = (1.0 / np.maximum(deg, 1))[core * dst_shard + loc]
    idxv = s_pad - bases[k]
    assert idxv.min() >= -RBASE and idxv.max() < RBASE

    dr_a = np.full((C, BLK, T), -1.0, np.float32)
    w_a = np.zeros((C, BLK, T), np.float32)
    ix_a = np.zeros((C, BLK, T), np.int16)
    dr_a[c_idx, p_idx, t_idx] = drel[order]
    w_a[c_idx, p_idx, t_idx] = w_e[order]
    ix_a[c_idx, p_idx, t_idx] = idxv[order]

    # the HW skips a trailing run of negative idxs in a gather stream; since
    # instructions may be split at any tile boundary, force slot 127 of every
    # tile to hold a non-negative idx (swap with the first such slot)
    for c in range(C):
        for t in np.nonzero(ix_a[c, BLK - 1, :] < 0)[0]:
            p = int(np.argmax(ix_a[c, :, t] >= 0))
            assert ix_a[c, p, t] >= 0, "all-negative tile"
            for arr in (ix_a, dr_a, w_a):
                tmp = arr[c, BLK - 1, t].copy()
                arr[c, BLK - 1, t] = arr[c, p, t]
                arr[c, p, t] = tmp

    # tile dst windows (union over cores); empty tiles -> [0,1)
    real = w_a > 0
    drm = np.where(real, dr_a, 1e9).min(axis=(0, 1))
    drx = np.where(real, dr_a, -1e9).max(axis=(0, 1))
    lo_t = np.where(drm > drx, 0, drm).astype(np.int64)
    hi_t = np.where(drm > drx, 1, drx + 1).astype(np.int64)

    groups, instrs = [], []
    for gg in range(ngrp):
        t_lo = int(tile_of_cell[gg, 0, 0])
        ins_g = []
        for kk in range(NR):
            nt_gk = int(nt_cell[gg, kk].sum())
            if nt_gk:
                ins_g.append((int(bases[kk]), int(tile_of_cell[gg, kk, 0]), nt_gk))
        t_hi = t_lo + sum(n for _, _, n in ins_g)
        groups.append((t_lo, t_hi))
        instrs.append(ins_g)

    chunk_tiles = [[] for _ in range(nch)]
    for gg in range(ngrp):
        for kk in range(NR):
            for c2 in range(GRP):
                ci = gg * GRP + c2
                if ci >= nch:
                    continue
                t0 = int(tile_of_cell[gg, kk, c2])
                for t in range(t0, t0 + int(nt_cell[gg, kk, c2])):
                    chunk_tiles[ci].append((t, int(lo_t[t]), int(hi_t[t])))

    # wrap-16 index layout: tile t slot p -> [16a + p%16, 8t + p//16]
    wr = ix_a.transpose(0, 2, 1).reshape(C, T, 8, 16)
    wr = wr.transpose(0, 3, 1, 2).reshape(C, 16, T * 8)
    ix_wrap = np.tile(wr, (1, 8, 1))

    grid = dict(T=T, ngrp=ngrp, groups=groups, instrs=instrs,
                chunk_tiles=chunk_tiles)
    return ix_wrap, dr_a, w_a, grid


def build(cfg, grids, b2val):
    H, FP, FS, L = cfg.H, cfg.FP, cfg.FS, cfg.L
    nc = bacc.Bacc(None, target_bir_lowering=False, num_devices=C,
                   num_swdge_queues=N_SWDGE_Q)

    def param(name, shape, dt):
        return nc.declare_dram_parameter(name, shape, dt, isOutput=False)

    xpt = param("xpt", [FP, cfg.NPP], BF)
    xst = param("xst", [FS, cfg.NSP], BF)
    eparams = {}
    for r in ("orb", "hst", "sib"):
        T = grids[r]["T"]
        eparams[r] = (
            param(f"{r}_ix", [BLK, 8 * T], I16),
            param(f"{r}_dr", [BLK, T], F32),
            param(f"{r}_w", [BLK, T], F32),
        )
    iota_p = param("iota", [128, SEG], F32)
    wp_p = param("wp", [FP, H], BF)
    bp_p = param("bp", [H, 1], F32)
    ws_p = param("ws", [FS, H], BF)
    bs_p = param("bs", [H, 1], F32)
    wstack_s_p = param("wstack_s", [L, 2 * H, H], BF)
    wstack_p_p = param("wstack_p", [L, 2 * H, H], BF)
    wr_p_p = param("wr_p", [L, H, H], BF)
    bias_s_p = param("bias_s", [L, H, 1], F32)
    bias_p_p = param("bias_p", [L, H, 1], F32)
    w1_p = param("w1", [H, H // 2], BF)
    b1_p = param("b1", [H // 2, 1], F32)
    w2_p = param("w2", [H // 2, 1], BF)
    out_p = nc.declare_dram_parameter("out", [1, cfg.NPP], F32, isOutput=True)

    pchunks = _chunks(cfg.PB)
    schunks = _chunks(cfg.SB)
    qctr = [0]

    def next_q():
        qctr[0] += 1
        return qctr[0] % N_SWDGE_Q

    with tile.TileContext(nc) as tc:
        with (
            tc.tile_pool(name="const", bufs=1) as cp,
            tc.tile_pool(name="dram", bufs=1, space="DRAM") as dp,
            tc.tile_pool(name="idx", bufs=4) as ip,
            tc.tile_pool(name="msg", bufs=2) as mp,
            tc.tile_pool(name="sel", bufs=2) as selp,
            tc.tile_pool(name="work", bufs=4) as wkp,
            tc.tile_pool(name="psum", bufs=1, space="PSUM") as pp,
        ):
            # ---- persistent DRAM state ----
            hp_tab = [
                dp.tile([cfg.NPT, EW], BF, addr_space="Shared",
                        tag=f"hp_tab{i}", name=f"hp_tab{i}")
                for i in range(L)
            ]
            hs_tab = [
                dp.tile([cfg.NST, EW], BF, addr_space="Shared",
                        tag=f"hs_tab{i}", name=f"hs_tab{i}")
                for i in range(L)
            ]
            xpT = [dp.tile([H, cfg.NPP], BF, tag=f"xpT{i}", name=f"xpT{i}")
                   for i in range(2)]
            xsT = [dp.tile([H, cfg.NSP], BF, tag=f"xsT{i}", name=f"xsT{i}")
                   for i in range(2)]
            hp_shard = dp.tile([cfg.NPP, EW], BF, tag="hp_shard")
            hs_shard = dp.tile([cfg.NSP, EW], BF, tag="hs_shard")

            # ---- consts ----
            iota_t = cp.tile([128, SEG], F32, tag="iota")
            nc.sync.dma_start(out=iota_t[:], in_=iota_p[:, :])
            wp_t = cp.tile([FP, H], BF, tag="wp")
            nc.sync.dma_start(out=wp_t[:], in_=wp_p[:, :])
            ws_t = cp.tile([FS, H], BF, tag="ws")
            nc.sync.dma_start(out=ws_t[:], in_=ws_p[:, :])
            bp_t = cp.tile([H, 1], F32, tag="bp")
            nc.sync.dma_start(out=bp_t[:], in_=bp_p[:, :])
            bs_t = cp.tile([H, 1], F32, tag="bs")
            nc.sync.dma_start(out=bs_t[:], in_=bs_p[:, :])
            w1_t = cp.tile([H, H // 2], BF, tag="w1")
            nc.sync.dma_start(out=w1_t[:], in_=w1_p[:, :])
            b1_t = cp.tile([H // 2, 1], F32, tag="b1")
            nc.sync.dma_start(out=b1_t[:], in_=b1_p[:, :])
            w2_t = cp.tile([H // 2, 1], BF, tag="w2")
            nc.sync.dma_start(out=w2_t[:], in_=w2_p[:, :])
            wstack_s_t, wstack_p_t, wr_p_t, bias_s_t, bias_p_t = [], [], [], [], []
            for l in range(L):
                t = cp.tile([2 * H, H], BF, tag=f"wss{l}")
                nc.sync.dma_start(out=t[:], in_=wstack_s_p[l, :, :])
                wstack_s_t.append(t)
                t = cp.tile([2 * H, H], BF, tag=f"wsp{l}")
                nc.sync.dma_start(out=t[:], in_=wstack_p_p[l, :, :])
                wstack_p_t.append(t)
                t = cp.tile([H, H], BF, tag=f"wrp{l}")
                nc.sync.dma_start(out=t[:], in_=wr_p_p[l, :, :])
                wr_p_t.append(t)
                t = cp.tile([H, 1], F32, tag=f"bss{l}")
                nc.sync.dma_start(out=t[:], in_=bias_s_p[l, :, :])
                bias_s_t.append(t)
                t = cp.tile([H, 1], F32, tag=f"bsp{l}")
                nc.sync.dma_start(out=t[:], in_=bias_p_p[l, :, :])
                bias_p_t.append(t)

            def allgather(shard, tab):
                nc.gpsimd.collective_compute(
                    "AllGather",
                    ALU.bypass,
                    replica_groups=[list(range(C))],
                    ins=[shard[:, :]],
                    outs=[tab[:, :]],
                )

            def write_out_chunk(ob, c0, cw, nb, shard, xT_next):
                nc.sync.dma_start(out=xT_next[:, c0 : c0 + cw], in_=ob[:, :cw])
                for bi in range(nb):
                    eng = nc.sync if bi % 2 == 0 else nc.scalar
                    nm = wkp.tile([128, H], BF, tag="nm", bufs=6)
                    eng.dma_start_transpose(
                        out=nm[:], in_=ob[:, bi * 128 : (bi + 1) * 128]
                    )
                    r0 = c0 + bi * 128
                    eng.dma_start(out=shard[r0 : r0 + 128, 0:H], in_=nm[:])

            def load_group(rel, gg, table):
                """Load dr/w/ix spans and gather the group's messages."""
                gr = grids[rel]
                t_lo, t_hi = gr["groups"][gg]
                n = t_hi - t_lo
                ixp, drp, wp_ = eparams[rel]
                dr_t = ip.tile([BLK, n], F32, tag=f"{rel}_dr")
                nc.sync.dma_start(out=dr_t[:], in_=drp[:, t_lo:t_hi])
                w_t = ip.tile([BLK, n], F32, tag=f"{rel}_w")
                nc.sync.dma_start(out=w_t[:], in_=wp_[:, t_lo:t_hi])
                ix_t = ip.tile([BLK, 8 * n], I16, tag=f"{rel}_ix")
                nc.sync.dma_start(out=ix_t[:], in_=ixp[:, 8 * t_lo : 8 * t_hi])
                msg = mp.tile([128, n, EW], BF, tag=f"{rel}_msg", bufs=(2 if rel == "orb" else 3))
                for (base, t0, nt) in gr["instrs"][gg]:
                    for s0 in range(0, nt, 8):
                        sn = min(8, nt - s0)
                        o = t0 - t_lo + s0
                        nc.gpsimd.dma_gather(
                            out_ap=msg[:, o : o + sn, :],
                            in_ap=table[base:, :],
                            idxs_ap=ix_t[:, 8 * o : 8 * (o + sn)],
                            num_idxs=sn * BLK,
                            num_idxs_reg=sn * BLK,
                            elem_size=EW,
                            queue_num=next_q(),
                        )
                return (dr_t, w_t, msg, t_lo, gr)

            def emit_agg(buf, ci, cw, agg):
                dr_t, w_t, msg, t_lo, gr = buf
                tiles = gr["chunk_tiles"][ci]
                ntl = len(tiles)
                for j, (gt, lo, hi) in enumerate(tiles):
                    if j == 0:
                        lo, hi = 0, cw
                    wdt = hi - lo
                    o = gt - t_lo
                    sel = selp.tile([128, SEG], BF, tag="sel", name="sel")
                    nc.vector.tensor_scalar(
                        out=sel[:, :wdt],
                        in0=iota_t[:, lo:hi],
                        scalar1=dr_t[:, o : o + 1],
                        scalar2=w_t[:, o : o + 1],
                        op0=ALU.is_equal,
                        op1=ALU.mult,
                    )
                    nc.tensor.matmul(
                        out=agg[:, lo:hi],
                        lhsT=msg[:, o : o + 1, 0:H].opt(),
                        rhs=sel[:, :wdt],
                        start=(j == 0),
                        stop=(j == ntl - 1),
                    )

            # =================== input projection ===================
            for (b0, nb) in pchunks:
                cw = nb * BLK
                c0 = b0 * BLK
                xp = wkp.tile([FP, SEG], BF, tag="xp")
                nc.sync.dma_start(out=xp[:, :cw], in_=xpt[:, c0 : c0 + cw])
                po = pp.tile([H, SEG], F32, tag="out", bufs=2)
                nc.tensor.matmul(
                    out=po[:, :cw], lhsT=wp_t[:], rhs=xp[:, :cw],
                    start=True, stop=True
                )
                ob = wkp.tile([H, SEG], BF, tag="ob")
                nc.scalar.activation(
                    out=ob[:, :cw], in_=po[:, :cw], func=AF.Relu,
                    bias=bp_t[:], scale=1.0
                )
                write_out_chunk(ob, c0, cw, nb, hp_shard, xpT[0])
            for (b0, nb) in schunks:
                cw = nb * BLK
                c0 = b0 * BLK
                xs = wkp.tile([FS, SEG], BF, tag="xs")
                nc.sync.dma_start(out=xs[:, :cw], in_=xst[:, c0 : c0 + cw])
                po = pp.tile([H, SEG], F32, tag="out", bufs=2)
                nc.tensor.matmul(
                    out=po[:, :cw], lhsT=ws_t[:], rhs=xs[:, :cw],
                    start=True, stop=True
                )
                ob = wkp.tile([H, SEG], BF, tag="ob")
                nc.scalar.activation(
                    out=ob[:, :cw], in_=po[:, :cw], func=AF.Relu,
                    bias=bs_t[:], scale=1.0
                )
                write_out_chunk(ob, c0, cw, nb, hs_shard, xsT[0])
            allgather(hp_shard, hp_tab[0])
            allgather(hs_shard, hs_tab[0])

            # =================== SAGE layers ===================
            for l in range(L):
                rp, wpar = l % 2, (l + 1) % 2
                rv, wv = l, l + 1
                # ---- stars (skip at last layer: no consumer) ----
                if l < L - 1:
                    for gg in range(grids["orb"]["ngrp"]):
                        buf_o = load_group("orb", gg, hp_tab[rv])
                        for ci in range(gg * GRP,
                                        min((gg + 1) * GRP, len(schunks))):
                            b0, nb = schunks[ci]
                            cw = nb * BLK
                            c0 = b0 * BLK
                            agg = pp.tile([H, SEG], F32, tag="agg_a", bufs=3)
                            emit_agg(buf_o, ci, cw, agg)
                            stacked = wkp.tile([2 * H, SEG], BF, tag="stacked")
                            nc.scalar.activation(
                                out=stacked[0:H, :cw], in_=agg[:, :cw],
                                func=AF.Copy
                            )
                            nc.sync.dma_start(
                                out=stacked[H : 2 * H, :cw],
                                in_=xsT[rp][:, c0 : c0 + cw]
                            )
                            po = pp.tile([H, SEG], F32, tag="out", bufs=2)
                            nc.tensor.matmul(
                                out=po[:, :cw],
                                lhsT=wstack_s_t[l][:],
                                rhs=stacked[:, :cw],
                                start=True,
                                stop=True,
                            )
                            ob = wkp.tile([H, SEG], BF, tag="ob")
                            nc.scalar.activation(
                                out=ob[:, :cw], in_=po[:, :cw], func=AF.Relu,
                                bias=bias_s_t[l][:], scale=1.0,
                            )
                            write_out_chunk(ob, c0, cw, nb, hs_shard, xsT[wpar])
                # ---- planets ----
                for gg in range(grids["sib"]["ngrp"]):
                    buf_h = load_group("hst", gg, hs_tab[rv])
                    buf_s = load_group("sib", gg, hp_tab[rv])
                    for ci in range(gg * GRP, min((gg + 1) * GRP, len(pchunks))):
                        b0, nb = pchunks[ci]
                        cw = nb * BLK
                        c0 = b0 * BLK
                        agg_h = pp.tile([H, SEG], F32, tag="agg_a", bufs=3)
                        emit_agg(buf_h, ci, cw, agg_h)
                        agg_s = pp.tile([H, SEG], F32, tag="agg_b", bufs=3)
                        emit_agg(buf_s, ci, cw, agg_s)
                        stacked = wkp.tile([2 * H, SEG], BF, tag="stacked")
                        nc.scalar.activation(
                            out=stacked[0:H, :cw], in_=agg_h[:, :cw], func=AF.Copy
                        )
                        nc.scalar.activation(
                            out=stacked[H : 2 * H, :cw], in_=agg_s[:, :cw],
                            func=AF.Copy
                        )
                        xt = wkp.tile([H, SEG], BF, tag="xt")
                        nc.sync.dma_start(
                            out=xt[:, :cw], in_=xpT[rp][:, c0 : c0 + cw]
                        )
                        po = pp.tile([H, SEG], F32, tag="out", bufs=2)
                        nc.tensor.matmul(
                            out=po[:, :cw],
                            lhsT=wstack_p_t[l][:],
                            rhs=stacked[:, :cw],
                            start=True,
                            stop=False,
                        )
                        nc.tensor.matmul(
                            out=po[:, :cw], lhsT=wr_p_t[l][:], rhs=xt[:, :cw],
                            start=False, stop=True,
                        )
                        ob = wkp.tile([H, SEG], BF, tag="ob")
                        nc.scalar.activation(
                            out=ob[:, :cw], in_=po[:, :cw], func=AF.Relu,
                            bias=bias_p_t[l][:], scale=1.0,
                        )
                        if l < L - 1:
                            write_out_chunk(ob, c0, cw, nb, hp_shard, xpT[wpar])
                        else:
                            prt = pp.tile([H, SEG], F32, tag="agg_a", bufs=3)
                            pr = prt[0 : H // 2, :]
                            nc.tensor.matmul(
                                out=pr[:, :cw], lhsT=w1_t[:], rhs=ob[:, :cw],
                                start=True, stop=True
                            )
                            r1 = wkp.tile([H // 2, SEG], BF, tag="r1sb")
                            nc.scalar.activation(
                                out=r1[:, :cw], in_=pr[:, :cw], func=AF.Relu,
                                bias=b1_t[:], scale=1.0,
                            )
                            pyt = pp.tile([H, SEG], F32, tag="agg_b", bufs=3)
                            py = pyt[0:1, :]
                            nc.tensor.matmul(
                                out=py[:, :cw], lhsT=w2_t[:], rhs=r1[:, :cw],
                                start=True, stop=True
                            )
                            ysb = wkp.tile([1, SEG], F32, tag="ysb")
                            nc.vector.tensor_scalar_add(
                                out=ysb[:, :cw], in0=py[:, :cw],
                                scalar1=float(b2val)
                            )
                            nc.sync.dma_start(
                                out=out_p[0:1, c0 : c0 + cw], in_=ysb[:, :cw]
                            )
                if l < L - 1:
                    allgather(hp_shard, hp_tab[wv])
                    allgather(hs_shard, hs_tab[wv])

    nc.finalize()
    return nc


def _prep_all(inputs, cfg):
    f32 = np.float32
    xp = np.asarray(inputs["x_planet"], f32)
    xs = np.asarray(inputs["x_star"], f32)
    Wp = np.asarray(inputs["Wp"], f32)
    bp = np.asarray(inputs["bp"], f32)
    Ws = np.asarray(inputs["Ws"], f32)
    bs = np.asarray(inputs["bs"], f32)
    Wl = np.asarray(inputs["Wl"], f32)
    bl = np.asarray(inputs["bl"], f32)
    Wr = np.asarray(inputs["Wr"], f32)
    W1 = np.asarray(inputs["W1"], f32)
    b1 = np.asarray(inputs["b1"], f32)
    W2 = np.asarray(inputs["W2"], f32)
    b2 = np.asarray(inputs["b2"], f32)

    rels = {}
    grids = {}
    rels["orb"] = _prep_rel2(inputs["orbits_src"], inputs["orbits_dst"],
                             cfg.SP, cfg.NPP, cfg.SS, cfg.SB, cfg.NPT)
    rels["hst"] = _prep_rel2(inputs["hosts_src"], inputs["hosts_dst"],
                             cfg.SS, cfg.NSP, cfg.SP, cfg.PB, cfg.NST)
    rels["sib"] = _prep_rel2(inputs["sib_src"], inputs["sib_dst"],
                             cfg.SP, cfg.NPP, cfg.SP, cfg.PB, cfg.NPT)
    for name in ("orb", "hst", "sib"):
        grids[name] = rels[name][3]

    L, H = cfg.L, cfg.H
    wstack_s = np.stack([np.concatenate([Wl[l, 0], Wr[l, 0]], 0) for l in range(L)])
    wstack_p = np.stack(
        [np.concatenate([0.5 * Wl[l, 1], 0.5 * Wl[l, 2]], 0) for l in range(L)]
    )
    wr_p = np.stack([0.5 * (Wr[l, 1] + Wr[l, 2]) for l in range(L)])
    bias_s = np.stack([bl[l, 0][:, None] for l in range(L)])
    bias_p = np.stack([0.5 * (bl[l, 1] + bl[l, 2])[:, None] for l in range(L)])
    iota = np.tile(np.arange(SEG, dtype=np.float32), (128, 1))

    common = {
        "iota": iota,
        "wp": Wp.astype(BF16), "bp": bp[:, None],
        "ws": Ws.astype(BF16), "bs": bs[:, None],
        "wstack_s": wstack_s.astype(BF16), "wstack_p": wstack_p.astype(BF16),
        "wr_p": wr_p.astype(BF16),
        "bias_s": bias_s, "bias_p": bias_p,
        "w1": W1.astype(BF16), "b1": b1[:, None], "w2": W2.astype(BF16),
    }
    in_maps = []
    for c in range(C):
        xpt_c = np.zeros((cfg.FP, cfg.NPP), BF16)
        xpt_c[:, : cfg.SP] = xp[c * cfg.SP : (c + 1) * cfg.SP].T.astype(BF16)
        xst_c = np.zeros((cfg.FS, cfg.NSP), BF16)
        xst_c[:, : cfg.SS] = xs[c * cfg.SS : (c + 1) * cfg.SS].T.astype(BF16)
        m = dict(common)
        m["xpt"] = xpt_c
        m["xst"] = xst_c
        for name in ("orb", "hst", "sib"):
            ix, dr, w, _ = rels[name]
            m[f"{name}_ix"] = ix[c]
            m[f"{name}_dr"] = dr[c]
            m[f"{name}_w"] = w[c]
        in_maps.append(m)
    return in_maps, grids, float(b2[0])


LAST_RESULT = None


def kernel(_cfg=None, _trace=False, **inputs):
    global LAST_RESULT
    cfg = _cfg or Cfg()
    in_maps, grids, b2val = _prep_all(inputs, cfg)
    nc = build(cfg, grids, b2val)
    res = run_bass_kernel_spmd(nc, in_maps, list(range(C)), trace=_trace)
    LAST_RESULT = res
    out = np.concatenate(
        [res.results[c]["out"][0, : cfg.SP] for c in range(C)]
    ).astype(np.float32)
    return out
